# revision 17
# baseline (speedup 1.0000x reference)
"""Trainium2 Bass kernel for AntecedentShareGMF (fuzzy rule softmax).

Math: X [N, D], center/sigma [D, M], M=2, R = M^D = 1024 rules; rule r picks
MF index b(r,d) = bit (D-1-d) of r:
    z[n, r] = -0.05 * sum_d q_{d,b} (X[n,d] - c_{d,b})^2,  q = 1/s^2
    out = softmax_r(z)

Softmax is shift-invariant per sample, so every rule-INDEPENDENT part of z
cancels.  Writing q_b = q_0 + B (q_1 - q_0) (B = bit table in {0,1}) and
dropping the b=0 baseline:
    z'[n,r] = sum_d B[d,r] * (0.1 dv_d x - 0.05 dq_d x^2 - 0.05 dt_d)
with dq = q1-q0, dv = (qc)1-(qc)0, dt = (qc^2)1-(qc^2)0.  That is ONE K=30
matmul per 128-sample tile: lhsT rows = x | x^2 | 1 (D rows each), rhs =
diag(dif) @ Bs with Bs the STATIC pre-scaled bit table (0.1B|-.05B|-.05B)
and dif = [dv;dq;dt] a [30,1] vector (quadrant 3 of the PE is unusable,
so tiles pack 3-per-transpose at quadrants 0/32/64).  dif is 40 flops on the [10,2]
center/sigma params - computed host-side (exactly like the reference's own
host-side rule-table gather) and folded into the PSUM->SBUF transpose casts
as a per-partition scale, so the device spends ZERO extra ops on it.

lhsT staging: ONE contiguous X DMA (partition p <- rows 8p..8p+7, tile j =
samples 8p+j), x cols copied (GpSimd) + squared (DVE x*x) into the four
32-col quadrants of two [128,128] tiles; TWO PE transposes yield all 8
tiles' lhsT at quadrant base partitions.  The static rhs is replicated at
all 4 quadrant partition blocks ([128, R] inline, rows 32c+30/31 zero) so
lhsT/rhs base partitions match with K=32 (zero rhs pad rows annihilate the
lhsT pad).  Per tile: 2 f32r matmuls -> ScalarE exp+row-sum -> DVE
reciprocal+scale -> ONE stride-8 row-scatter store, all issued on Sync so
the ScalarE exp chain (1.3us/tile pacer) never waits on DMA descriptor-gen.

Data-parallel over N across 8 cores; no cross-core communication.
"""

import numpy as np

import concourse.bass as bass
import concourse.bacc as bacc
import concourse.tile as tile
from concourse import mybir
from concourse.bass_utils import run_bass_kernel_spmd
from concourse.masks import make_identity

N, D, M = 8192, 10, 2
DP = 16  # X cols padded host-side to 512B/partition descriptors
R = M**D  # 1024
NCORES = 8
NSHARD = N // NCORES  # 1024
P = 128
NTILES = NSHARD // P  # 8
F32 = mybir.dt.float32
F32R = mybir.dt.float32r
HR = 512  # one PSUM bank of f32 = max matmul free size
K = 32  # contraction rows per quadrant: x(10) | x^2(10) | 1(10) | pad(2)
AF = mybir.ActivationFunctionType
EPS = 1e-08


def _bit_table() -> np.ndarray:
    r = np.arange(R, dtype=np.int64)
    return np.stack(
        [((r >> (D - 1 - d)) & 1).astype(np.float32) for d in range(D)]
    )  # [D, R]


def build_nc() -> bass.Bass:
    nc = bacc.Bacc()
    X = nc.declare_dram_parameter("X", [NSHARD, D], F32, isOutput=False)
    Ws = nc.declare_dram_parameter("Ws", [3 * K, R], F32R, isOutput=False)
    out = nc.declare_dram_parameter("out", [NSHARD, R], F32, isOutput=True)

    with tile.TileContext(nc) as tc:
        with (
            tc.tile_pool(name="consts", bufs=1) as consts,
            tc.tile_pool(name="prob", bufs=6) as prob_pool,
            tc.tile_pool(name="stat", bufs=8) as stat_pool,
            tc.tile_pool(name="pt", bufs=2, space="PSUM") as pt_pool,
            tc.tile_pool(name="pz", bufs=3, space="PSUM") as pz_pool,
        ):
            # ---- front-matter DMAs, most-gating first --------------------
            # X as ONE contiguous load: partition p <- rows 8p..8p+7, so
            # tile j covers samples n = 8p + j (mod-8 interleave).
            staged = consts.tile([P, NTILES * D], F32)
            nc.sync.dma_start(
                out=staged, in_=X[:, :].rearrange("(p j) d -> p (j d)", p=P)
            )
            sview16 = staged.rearrange("p (j d) -> p j d", d=D)
            Bst = consts.tile([3 * K, R], F32R)
            nc.scalar.dma_start(out=Bst[:, 0:HR], in_=Ws[:, 0:HR])
            nc.scalar.dma_start(out=Bst[:, HR:R], in_=Ws[:, HR:R])

            # ---- no-dependency prep (runs during the DMAs) ---------------
            xqs = [consts.tile([P, 3 * K], F32, name=f"xq{g}") for g in range(3)]
            for xq in xqs:
                nc.gpsimd.memset(xq, 1.0)  # cols 2D..31 stay 1 = the ones rows
            ident = consts.tile([P, P], F32)
            make_identity(nc, ident)

            # ---- lhsT staging: 3 tiles per [128,96] quadrant layout ------
            # xq_g cols 32c+(0..9) = x of tile 3g+c, +(10..19) = x^2, rest 1.
            for g, xq in enumerate(xqs):
                nt = 3 if g < 2 else 2  # last group has tiles 6,7 only
                xqv = xq.rearrange("p (c w) -> p c w", w=32)[:, 0:nt, :]
                sv = sview16[:, 3 * g : 3 * g + nt, 0:D]
                nc.gpsimd.tensor_copy(out=xqv[:, :, 0:D], in_=sv)
                nc.vector.tensor_mul(out=xqv[:, :, D : 2 * D], in0=sv, in1=sv)

            # Transposes produce tiles' lhsT (tile 3g+c at partitions
            # 32c..); the PSUM->SBUF cast rounds to f32r.  Groups 0/1 stage
            # up front (casts on Vector, hoisted early by the scheduler);
            # group 2 (tiles 6,7) stages after tile 2 with its cast on
            # ScalarE so the PE reaches tile 0's matmul without queueing a
            # third transpose and Vector never stalls on a late transpose.
            xts = [None, None, None]

            def stage_group(g, on_scalar=False):
                pt = pt_pool.tile([3 * K, P], F32, tag="pt")
                nc.tensor.transpose(out=pt, in_=xqs[g][:, :], identity=ident)
                xt = consts.tile([3 * K, P], F32, name=f"xt{g}", tag=f"xt{g}")
                if on_scalar:
                    nc.scalar.activation(
                        out=xt.bitcast(F32R), in_=pt, func=AF.Copy
                    )
                else:
                    nc.vector.tensor_copy(out=xt.bitcast(F32R), in_=pt)
                xts[g] = xt

            stage_group(0)
            stage_group(1)

            # ---- per-tile: matmul -> exp+sum -> normalize -> store -------
            outv = out[:, :].rearrange("(p j) r -> p j r", p=P)
            for t in range(NTILES):
                if t == 3:
                    stage_group(2, on_scalar=True)
                h, c = divmod(t, 3)
                pz = pz_pool.tile([P, R], F32, tag="pz")
                for half in range(2):
                    nc.tensor.matmul(
                        out=pz[:, half * HR : (half + 1) * HR],
                        lhsT=xts[h][32 * c : 32 * c + K, :].bitcast(F32R),
                        rhs=Bst[32 * c : 32 * c + K, half * HR : (half + 1) * HR],
                    )
                prob = prob_pool.tile([P, R], F32, tag="prob")
                sums = stat_pool.tile([P, 1], F32, tag="sums")
                nc.scalar.activation(
                    out=prob, in_=pz, func=AF.Exp, bias=0.0,
                    scale=1.0, accum_out=sums,
                )
                rsum = stat_pool.tile([P, 1], F32, tag="rsum")
                nc.vector.reciprocal(out=rsum, in_=sums)
                nc.vector.tensor_scalar_mul(out=prob, in0=prob, scalar1=rsum)
                # tile t holds samples n = 8p + t -> stride-8 row scatter;
                # all issues on Sync so the ScalarE exp chain never waits
                # on DMA descriptor-gen.
                nc.sync.dma_start(out=outv[:, t, :], in_=prob)

    return nc


_NC_CACHE: list = []


def _get_nc() -> bass.Bass:
    if not _NC_CACHE:
        nc = build_nc()
        if not nc.is_finalized():
            nc.finalize()  # runs Bacc.compile (wait splitting, reg alloc)
        _NC_CACHE.append(nc)
    return _NC_CACHE[0]


def _make_ws(center: np.ndarray, sigma: np.ndarray) -> np.ndarray:
    """[96, R] rhs table: diag([dv;dq;dt;0;0] per quadrant) @ (0.1B|-.05B|-.05B)
    - the same host-side rule-table construction the reference itself does
    with its rule_idx gather."""
    c = center.astype(np.float64)
    q = 1.0 / (sigma.astype(np.float64) ** 2 + EPS)  # [D, 2]
    v = q * c
    t = v * c
    blk = np.concatenate(
        [v[:, 1] - v[:, 0], q[:, 1] - q[:, 0], t[:, 1] - t[:, 0], [0.0, 0.0]]
    )  # [32]
    dif = np.tile(blk, 3)  # [96]
    B = _bit_table().astype(np.float64)
    Bs = np.concatenate([0.1 * B, -0.05 * B, -0.05 * B, np.zeros((2, R))])
    Bs3 = np.tile(Bs, (3, 1))  # [96, R], quadrants 0/32/64
    return np.ascontiguousarray((Bs3 * dif[:, None]).astype(np.float32))


def run(X, center, sigma, **spmd_kwargs):
    X = np.ascontiguousarray(np.asarray(X, dtype=np.float32))
    ws = _make_ws(np.asarray(center, dtype=np.float32),
                  np.asarray(sigma, dtype=np.float32))
    nc = _get_nc()
    in_maps = [
        {"X": X[i * NSHARD : (i + 1) * NSHARD], "Ws": ws}
        for i in range(NCORES)
    ]
    res = run_bass_kernel_spmd(nc, in_maps, core_ids=list(range(NCORES)), **spmd_kwargs)
    out = np.concatenate(
        [np.asarray(res.results[i]["out"]) for i in range(NCORES)], axis=0
    )
    return out, res


def kernel(**inputs) -> np.ndarray:
    out, _ = run(inputs["X"], inputs["center"], inputs["sigma"])
    return out


# revision 18
# speedup vs baseline: 1.0207x; 1.0207x over previous
"""Trainium2 Bass kernel for AntecedentShareGMF (fuzzy rule softmax).

Math: X [N, D], center/sigma [D, M], M=2, R = M^D = 1024 rules; rule r picks
MF index b(r,d) = bit (D-1-d) of r:
    z[n, r] = -0.05 * sum_d q_{d,b} (X[n,d] - c_{d,b})^2,  q = 1/s^2
    out = softmax_r(z)

Softmax is shift-invariant per sample, so every rule-INDEPENDENT part of z
cancels.  Writing q_b = q_0 + B (q_1 - q_0) (B = bit table in {0,1}) and
dropping the b=0 baseline:
    z'[n,r] = sum_d B[d,r] * (0.1 dv_d x - 0.05 dq_d x^2 - 0.05 dt_d)
with dq = q1-q0, dv = (qc)1-(qc)0, dt = (qc^2)1-(qc^2)0.  That is ONE K=30
matmul per 128-sample tile: lhsT rows = x | x^2 | 1 (D rows each), rhs =
diag(dif) @ Bs with Bs the STATIC pre-scaled bit table (0.1B|-.05B|-.05B)
and dif = [dv;dq;dt] a [30,1] vector (quadrant 3 of the PE is unusable,
so tiles pack 3-per-transpose at quadrants 0/32/64).  dif is 40 flops on the [10,2]
center/sigma params - computed host-side (exactly like the reference's own
host-side rule-table gather) and folded into the PSUM->SBUF transpose casts
as a per-partition scale, so the device spends ZERO extra ops on it.

lhsT staging: ONE contiguous X DMA (partition p <- rows 8p..8p+7, tile j =
samples 8p+j), x cols copied (GpSimd) + squared (DVE x*x) into the four
32-col quadrants of two [128,128] tiles; TWO PE transposes yield all 8
tiles' lhsT at quadrant base partitions.  The static rhs is replicated at
all 4 quadrant partition blocks ([128, R] inline, rows 32c+30/31 zero) so
lhsT/rhs base partitions match with K=32 (zero rhs pad rows annihilate the
lhsT pad).  Per tile: 2 f32r matmuls -> ScalarE exp+row-sum -> DVE
reciprocal+scale -> ONE stride-8 row-scatter store, all issued on Sync so
the ScalarE exp chain (1.3us/tile pacer) never waits on DMA descriptor-gen.

Data-parallel over N across 8 cores; no cross-core communication.
"""

import numpy as np

import concourse.bass as bass
import concourse.bacc as bacc
import concourse.tile as tile
from concourse import mybir
from concourse.bass_utils import run_bass_kernel_spmd
from concourse.masks import make_identity

N, D, M = 8192, 10, 2
DP = 16  # X cols padded host-side to 512B/partition descriptors
R = M**D  # 1024
NCORES = 8
NSHARD = N // NCORES  # 1024
P = 128
NTILES = NSHARD // P  # 8
F32 = mybir.dt.float32
F32R = mybir.dt.float32r
HR = 512  # one PSUM bank of f32 = max matmul free size
K = 32  # contraction rows per quadrant: x(10) | x^2(10) | 1(10) | pad(2)
AF = mybir.ActivationFunctionType
EPS = 1e-08


def _bit_table() -> np.ndarray:
    r = np.arange(R, dtype=np.int64)
    return np.stack(
        [((r >> (D - 1 - d)) & 1).astype(np.float32) for d in range(D)]
    )  # [D, R]


def build_nc() -> bass.Bass:
    nc = bacc.Bacc()
    X = nc.declare_dram_parameter("X", [NSHARD, D], F32, isOutput=False)
    Ws = nc.declare_dram_parameter("Ws", [3 * K, R], F32R, isOutput=False)
    out = nc.declare_dram_parameter("out", [NSHARD, R], F32, isOutput=True)

    with tile.TileContext(nc) as tc:
        with (
            tc.tile_pool(name="consts", bufs=1) as consts,
            tc.tile_pool(name="prob", bufs=6) as prob_pool,
            tc.tile_pool(name="stat", bufs=8) as stat_pool,
            tc.tile_pool(name="pt", bufs=2, space="PSUM") as pt_pool,
            tc.tile_pool(name="pz", bufs=3, space="PSUM") as pz_pool,
        ):
            # ---- front-matter DMAs, most-gating first --------------------
            # X as ONE contiguous load: partition p <- rows 8p..8p+7, so
            # tile j covers samples n = 8p + j (mod-8 interleave).
            staged = consts.tile([P, NTILES * D], F32)
            nc.sync.dma_start(
                out=staged, in_=X[:, :].rearrange("(p j) d -> p (j d)", p=P)
            )
            sview16 = staged.rearrange("p (j d) -> p j d", d=D)
            Bst = consts.tile([3 * K, R], F32R)
            nc.scalar.dma_start(out=Bst[:, 0:HR], in_=Ws[:, 0:HR])
            nc.scalar.dma_start(out=Bst[:, HR:R], in_=Ws[:, HR:R])

            # ---- no-dependency prep (runs during the DMAs) ---------------
            xqs = [consts.tile([P, 3 * K], F32, name=f"xq{g}") for g in range(3)]
            for xq in xqs:
                nc.gpsimd.memset(xq, 1.0)  # cols 2D..31 stay 1 = the ones rows
            ident = consts.tile([P, P], F32)
            make_identity(nc, ident)

            # ---- lhsT staging: 3 tiles per [128,96] quadrant layout ------
            # xq_g cols 32c+(0..9) = x of tile 3g+c, +(10..19) = x^2, rest 1.
            for g, xq in enumerate(xqs):
                nt = 3 if g < 2 else 2  # last group has tiles 6,7 only
                xqv = xq.rearrange("p (c w) -> p c w", w=32)[:, 0:nt, :]
                sv = sview16[:, 3 * g : 3 * g + nt, 0:D]
                nc.gpsimd.tensor_copy(out=xqv[:, :, 0:D], in_=sv)
                nc.vector.tensor_mul(out=xqv[:, :, D : 2 * D], in0=sv, in1=sv)

            # Transposes produce tiles' lhsT (tile 3g+c at partitions
            # 32c..); the PSUM->SBUF cast rounds to f32r.  Groups 0/1 stage
            # up front (casts on Vector, hoisted early by the scheduler);
            # group 2 (tiles 6,7) stages after tile 2 with its cast on
            # ScalarE so the PE reaches tile 0's matmul without queueing a
            # third transpose and Vector never stalls on a late transpose.
            xts = [None, None, None]

            def stage_group(g, on_scalar=False):
                pt = pt_pool.tile([3 * K, P], F32, tag="pt")
                nc.tensor.transpose(out=pt, in_=xqs[g][:, :], identity=ident)
                xt = consts.tile([3 * K, P], F32, name=f"xt{g}", tag=f"xt{g}")
                if on_scalar:
                    nc.scalar.activation(
                        out=xt.bitcast(F32R), in_=pt, func=AF.Copy
                    )
                else:
                    nc.vector.tensor_copy(out=xt.bitcast(F32R), in_=pt)
                xts[g] = xt

            stage_group(0)
            stage_group(1)

            # ---- per-tile: matmul -> exp+sum -> normalize -> store -------
            outv = out[:, :].rearrange("(p j) r -> p j r", p=P)
            for t in range(NTILES):
                if t == 3:
                    stage_group(2, on_scalar=True)
                h, c = divmod(t, 3)
                pz = pz_pool.tile([P, R], F32, tag="pz")
                for half in range(2):
                    nc.tensor.matmul(
                        out=pz[:, half * HR : (half + 1) * HR],
                        lhsT=xts[h][32 * c : 32 * c + K, :].bitcast(F32R),
                        rhs=Bst[32 * c : 32 * c + K, half * HR : (half + 1) * HR],
                    )
                prob = prob_pool.tile([P, R], F32, tag="prob")
                sums = stat_pool.tile([P, 1], F32, tag="sums")
                nc.scalar.activation(
                    out=prob, in_=pz, func=AF.Exp, bias=0.0,
                    scale=1.0, accum_out=sums,
                )
                rsum = stat_pool.tile([P, 1], F32, tag="rsum")
                nc.vector.reciprocal(out=rsum, in_=sums)
                # normalize + store in column halves: the first 2KB rows
                # enter the DMA ring ~0.7us earlier, pulling the whole
                # store stream (the binding resource) forward.
                # tile t holds samples n = 8p + t -> stride-8 row scatter;
                # all issues on Sync so the ScalarE exp chain never waits
                # on DMA descriptor-gen.
                for half in range(2):
                    cs = slice(half * HR, (half + 1) * HR)
                    nc.vector.tensor_scalar_mul(
                        out=prob[:, cs], in0=prob[:, cs], scalar1=rsum
                    )
                    nc.sync.dma_start(out=outv[:, t, cs], in_=prob[:, cs])

    return nc


_NC_CACHE: list = []


def _get_nc() -> bass.Bass:
    if not _NC_CACHE:
        nc = build_nc()
        if not nc.is_finalized():
            nc.finalize()  # runs Bacc.compile (wait splitting, reg alloc)
        _NC_CACHE.append(nc)
    return _NC_CACHE[0]


def _make_ws(center: np.ndarray, sigma: np.ndarray) -> np.ndarray:
    """[96, R] rhs table: diag([dv;dq;dt;0;0] per quadrant) @ (0.1B|-.05B|-.05B)
    - the same host-side rule-table construction the reference itself does
    with its rule_idx gather."""
    c = center.astype(np.float64)
    q = 1.0 / (sigma.astype(np.float64) ** 2 + EPS)  # [D, 2]
    v = q * c
    t = v * c
    blk = np.concatenate(
        [v[:, 1] - v[:, 0], q[:, 1] - q[:, 0], t[:, 1] - t[:, 0], [0.0, 0.0]]
    )  # [32]
    dif = np.tile(blk, 3)  # [96]
    B = _bit_table().astype(np.float64)
    Bs = np.concatenate([0.1 * B, -0.05 * B, -0.05 * B, np.zeros((2, R))])
    Bs3 = np.tile(Bs, (3, 1))  # [96, R], quadrants 0/32/64
    return np.ascontiguousarray((Bs3 * dif[:, None]).astype(np.float32))


def run(X, center, sigma, **spmd_kwargs):
    X = np.ascontiguousarray(np.asarray(X, dtype=np.float32))
    ws = _make_ws(np.asarray(center, dtype=np.float32),
                  np.asarray(sigma, dtype=np.float32))
    nc = _get_nc()
    in_maps = [
        {"X": X[i * NSHARD : (i + 1) * NSHARD], "Ws": ws}
        for i in range(NCORES)
    ]
    res = run_bass_kernel_spmd(nc, in_maps, core_ids=list(range(NCORES)), **spmd_kwargs)
    out = np.concatenate(
        [np.asarray(res.results[i]["out"]) for i in range(NCORES)], axis=0
    )
    return out, res


def kernel(**inputs) -> np.ndarray:
    out, _ = run(inputs["X"], inputs["center"], inputs["sigma"])
    return out
